# revision 13
# baseline (speedup 1.0000x reference)
"""KANLinear TRN2 Bass kernel (8-core SPMD, token-data-parallel).

Math (matches the jax reference, up to fp rounding):
  y[b,o] = silu(x)[b,:] @ scale_base.T  +  sum_{i,g} B_g(x[b,i]) * w[o,i,g]
with cubic B-spline bases on the uniform grid t_j = -1.75 + 0.25*j.

Basis evaluation uses the bounded symmetric form (exact identity, no
catastrophic cancellation, so the whole chain runs in fp16):
  a   = |4x + 5 - g|          (ACT Abs, f32 affine)
  m   = relu(2 - a) = -(min(a,2) - 2)
  n   = relu(1 - a) = -(min(a,1) - 1)
  6*B_g(x) = m^3 - 4*n^3
On device we carry u = -m, w = -n (one tensor_scalar each, 4x fp16 DVE
mode) and emit Bneg = -(6B) = m2*u - (Square(2w))*w with m2 = u*u.
Bases are quantized to fp8e4 and the spline einsum runs as fp8 DoubleRow
matmuls (K=256/instr, 2x PE rate); weights are host-scaled by 512/6 with
sign folded (-coeff), and the PSUM drain applies the 1/512.
The silu base matmul stays bf16 (it carries ~96% of the output norm) and
accumulates into the same PSUM banks with sbt host-scaled by 512; it is
issued FIRST per half so the PE tail is only the last spline k-tiles.

In-dim tiles are processed in PAIRS: the Abs/Silu ACT ops run once per
pair over [128, 1024] (halves ACT op overhead), and the two odd g=10
channels of a pair form one extra DoubleRow k-pair (4D b8/w8 tiles), so
all spline matmuls run in DoubleRow mode. SBUF pressure is handled by
rotating {a, T1} through one bufs=2 pool (so the next pair's Abs only
waits on the clamps of the current pair) and {m2, T2} through a bufs=1
pool (all strictly write-after-read safe, same-engine ordered).
"""

import numpy as np
import ml_dtypes

import concourse.bass as bass
import concourse.mybir as mybir
import concourse.tile as tile
from concourse import bacc
from concourse.alu_op_type import AluOpType as A
from concourse.bass_utils import run_bass_kernel_spmd

AF = mybir.ActivationFunctionType
F32 = mybir.dt.float32
F16 = mybir.dt.float16
BF16 = mybir.dt.bfloat16
F8 = mybir.dt.float8e4
DR = mybir.MatmulPerfMode.DoubleRow

# problem constants (hardcoded per the task contract)
TOKENS, IN_DIM, OUT_DIM = 8192, 1024, 1024
NB = 11  # cubic B-spline bases per input dim (grid_size + k)
N_CORES = 8
TPC = TOKENS // N_CORES  # tokens per core (1024)
HALF = 512  # tokens per processing chunk (PSUM-bank limited)
NIT = IN_DIM // 128  # in-dim tiles (8)
NPAIR = NIT // 2  # in-dim tile pairs (4)
M_TILES = HALF // 128  # token tiles per half (4)
N_OC = OUT_DIM // 512  # out-dim chunks (2)
WOC = NB * 512  # weight free size per (it, oc) chunk (5632)
WSCALE = 512.0  # fp8 weight scale (power of 2; undone in the PSUM drain)
NCONV_ACT = 7  # basis channels converted fp16->fp8 on ACT (rest on DVE)

_CACHED = None


def _silu_matmuls(nc, psums, silu_pair, sbtts, first_pair):
    for j in range(2):
        for oc in range(N_OC):
            for m in range(M_TILES):
                nc.tensor.matmul(
                    psums[oc][m],
                    lhsT=silu_pair[:, j, m * 128:m * 128 + 128],
                    rhs=sbtts[j][:, oc * 512:(oc + 1) * 512],
                    start=(first_pair and j == 0), stop=False)


def _spline_matmuls(nc, psums, b8, w8t, oc, last_pair):
    # b8: [128, 11, 2, 512] (channel, it-in-pair, token)
    # w8t: [128, 2, 11, 512] (it-in-pair, channel, out-chunk)
    for m in range(M_TILES):
        ms = slice(m * 128, m * 128 + 128)
        for j in range(2):
            for gp in range(5):
                nc.tensor.matmul(
                    psums[oc][m],
                    lhsT=b8[:, 2 * gp:2 * gp + 2, j, ms],
                    rhs=w8t[:, j, 2 * gp:2 * gp + 2, :],
                    start=False, stop=False, perf_mode=DR)
        # the two g=10 channels of the pair form one k-pair
        nc.tensor.matmul(
            psums[oc][m], lhsT=b8[:, 10, 0:2, ms],
            rhs=w8t[:, 0:2, 10, :],
            start=False, stop=last_pair, perf_mode=DR)


def _build_bass():
    nc = bacc.Bacc("TRN2", target_bir_lowering=False, debug=False,
                   num_devices=N_CORES)
    xt = nc.declare_dram_parameter("xt", [IN_DIM, TPC], F16, isOutput=False)
    w8 = nc.declare_dram_parameter("w8", [IN_DIM, NB * OUT_DIM], F8,
                                   isOutput=False)
    sbt = nc.declare_dram_parameter("sbt", [IN_DIM, OUT_DIM], BF16,
                                    isOutput=False)
    y = nc.declare_dram_parameter("y", [TPC, OUT_DIM], F32, isOutput=True)

    with tile.TileContext(nc) as tc:
        with (
            tc.tile_pool(name="xts", bufs=2) as xpool,
            tc.tile_pool(name="silu", bufs=2) as spool,
            tc.tile_pool(name="at", bufs=2) as atpool,    # a -> T1 rotation
            tc.tile_pool(name="mt", bufs=1) as mtpool,    # m2 -> T2 rotation
            tc.tile_pool(name="un", bufs=1) as upool,
            tc.tile_pool(name="n2", bufs=1) as n2pool,
            tc.tile_pool(name="wb", bufs=1) as wpool_,
            tc.tile_pool(name="bn", bufs=1) as bnpool,
            tc.tile_pool(name="b8b", bufs=2) as b8pool,
            tc.tile_pool(name="w8t", bufs=2) as w8pool,
            tc.tile_pool(name="sbts", bufs=2) as sbpool,
            tc.tile_pool(name="outs", bufs=1) as opool,
            tc.tile_pool(name="consts", bufs=1) as kpool,
            tc.tile_pool(name="psum", bufs=8, space="PSUM") as ppool,
        ):
            # per-channel bias constants for a_g = |4x + (5-g)|
            bias_tile = kpool.tile([128, NB + 1], F32, tag="bias")
            for g in range(NB):
                nc.vector.memset(bias_tile[:, g:g + 1], float(5 - g))
            nc.vector.memset(bias_tile[:, NB:NB + 1], 0.0)  # zero bias
            zb = bias_tile[:, NB:NB + 1]

            for half in range(2):
                t0 = half * HALF
                psums = [[ppool.tile([128, 512], F32, tag="ps",
                                     name=f"ps_{half}_{_oc}_{_m}")
                          for _m in range(M_TILES)] for _oc in range(N_OC)]
                for pair in range(NPAIR):
                    it0 = 2 * pair
                    # x for both its of the pair: [128, 2, 512] f32
                    xp = xpool.tile([128, 2, HALF], F16, tag="xt")
                    for j in range(2):
                        for q in range(2):
                            nc.sync.dma_start(
                                out=xp[:, j, q * 256:(q + 1) * 256],
                                in_=xt[(it0 + j) * 128:(it0 + j + 1) * 128,
                                       t0 + q * 256:t0 + (q + 1) * 256])
                    sp = spool.tile([128, 2, HALF], BF16, tag="silu")
                    nc.scalar.activation(sp, xp, AF.Silu)
                    sbtts = []
                    for j in range(2):
                        sbtt = sbpool.tile([128, OUT_DIM], BF16, tag="sbt")
                        nc.sync.dma_start(
                            out=sbtt,
                            in_=sbt[(it0 + j) * 128:(it0 + j + 1) * 128, :])
                        sbtts.append(sbtt)
                    _silu_matmuls(nc, psums, sp, sbtts, pair == 0)

                    # 11 abs channels for the pair -> [128, 11, 1024] fp16
                    a = atpool.tile([128, NB, 2 * HALF], F16, tag="at")
                    for g in range(NB):
                        nc.scalar.activation(a[:, g, :], xp, AF.Abs,
                                             bias=bias_tile[:, g:g + 1],
                                             scale=4.0)
                    # u = min(a,2)-2 = -m ; w = min(a,1)-1 = -n   (ts, 4x)
                    u = upool.tile([128, NB, 2 * HALF], F16, tag="un")
                    nc.vector.tensor_scalar(u, a, 2.0, 2.0, A.min, A.subtract)
                    w = wpool_.tile([128, NB, 2 * HALF], F16, tag="wb")
                    nc.vector.tensor_scalar(w, a, 1.0, 1.0, A.min, A.subtract)
                    # m2 = u*u ; T1 = m2*u = -m^3   (pure DVE, first)
                    m2 = mtpool.tile([128, NB, 2 * HALF], F16, tag="mt")
                    nc.vector.tensor_mul(m2, u, u)
                    T1 = atpool.tile([128, NB, 2 * HALF], F16, tag="at")
                    nc.vector.tensor_mul(T1, m2, u)
                    # n2q = (2w)^2 = 4n^2 (ACT) ; T2 = n2q*w = -4n^3
                    n2q = n2pool.tile([128, NB, 2 * HALF], F16, tag="n2")
                    nc.scalar.activation(n2q, w, AF.Square, bias=zb,
                                         scale=2.0)
                    T2 = mtpool.tile([128, NB, 2 * HALF], F16, tag="mt")
                    nc.vector.tensor_mul(T2, n2q, w)
                    # Bneg = T1 - T2 = -(6B); channels < NCONV_ACT go
                    # through fp16 + ACT convert, the rest are written
                    # straight to fp8 by the DVE subtract (1x mode).
                    b8 = b8pool.tile([128, NB, 2, HALF], F8, tag="b8")
                    Bneg = bnpool.tile([128, NCONV_ACT, 2 * HALF], F16,
                                       tag="bn")
                    nc.vector.tensor_sub(Bneg, T1[:, :NCONV_ACT, :],
                                         T2[:, :NCONV_ACT, :])
                    nc.scalar.activation(b8[:, :NCONV_ACT, :, :], Bneg,
                                         AF.Copy, bias=0.0, scale=1.0)
                    nc.vector.tensor_sub(b8[:, NCONV_ACT:, :, :],
                                         T1[:, NCONV_ACT:, :],
                                         T2[:, NCONV_ACT:, :])

                    # weights per oc chunk: [128, 2, 11, 512] fp8
                    for oc in range(N_OC):
                        w8t = w8pool.tile([128, 2, NB, 512], F8, tag="w8")
                        for j in range(2):
                            nc.sync.dma_start(
                                out=w8t[:, j, :, :],
                                in_=w8[(it0 + j) * 128:(it0 + j + 1) * 128,
                                       oc * WOC:(oc + 1) * WOC])
                        _spline_matmuls(nc, psums, b8, w8t, oc,
                                        pair == NPAIR - 1)
                # drain PSUM -> SBUF (undo WSCALE) -> HBM
                for oc in range(N_OC):
                    for m in range(M_TILES):
                        ot = opool.tile([128, 512], F32, tag="out")
                        nc.scalar.activation(ot, psums[oc][m], AF.Copy,
                                             bias=0.0, scale=1.0 / WSCALE)
                        r0 = t0 + m * 128
                        nc.sync.dma_start(
                            out=y[r0:r0 + 128, oc * 512:(oc + 1) * 512],
                            in_=ot)
    nc.compile()
    return nc


def _prepare_inputs(x, coeff, scale_base, scale_spline):
    x = np.asarray(x, dtype=np.float32)
    coeff = np.asarray(coeff, dtype=np.float32)
    scale_base = np.asarray(scale_base, dtype=np.float32)
    ss = float(np.asarray(scale_spline).reshape(-1)[0])
    # w8[i, oc*5632 + g*512 + o] = -coeff[oc*512+o, i, g] * ss * WSCALE/6
    w8 = (coeff * (-ss * WSCALE / 6.0)).transpose(1, 2, 0)  # [i, g, o]
    w8 = w8.reshape(IN_DIM, NB, N_OC, 512).transpose(0, 2, 1, 3)
    w8 = np.ascontiguousarray(w8).reshape(IN_DIM, NB * OUT_DIM)
    w8 = w8.astype(ml_dtypes.float8_e4m3)
    sbt = np.ascontiguousarray(scale_base.T * WSCALE).astype(
        ml_dtypes.bfloat16)
    in_maps = []
    for c in range(N_CORES):
        xt = np.ascontiguousarray(x[c * TPC:(c + 1) * TPC, :].T).astype(
            np.float16)
        in_maps.append({"xt": xt, "w8": w8, "sbt": sbt})
    return in_maps


def _get_bass():
    global _CACHED
    if _CACHED is None:
        _CACHED = _build_bass()
    return _CACHED


def run(inputs, trace=False, **kw):
    nc = _get_bass()
    in_maps = _prepare_inputs(inputs["x"], inputs["coeff"],
                              inputs["scale_base"], inputs["scale_spline"])
    res = run_bass_kernel_spmd(nc, in_maps, list(range(N_CORES)),
                               trace=trace, **kw)
    y = np.concatenate([np.asarray(res.results[c]["y"])
                        for c in range(N_CORES)], axis=0)
    return np.ascontiguousarray(y.astype(np.float32)), res


def kernel(x, grid, coeff, scale_base, scale_spline):
    y, _ = run({"x": x, "grid": grid, "coeff": coeff,
                "scale_base": scale_base, "scale_spline": scale_spline})
    return y


# revision 15
# speedup vs baseline: 1.0526x; 1.0526x over previous
"""KANLinear TRN2 Bass kernel (8-core SPMD, token-data-parallel).

Math (matches the jax reference, up to fp rounding):
  y[b,o] = silu(x)[b,:] @ scale_base.T  +  sum_{i,g} B_g(x[b,i]) * w[o,i,g]
with cubic B-spline bases on the uniform grid t_j = -1.75 + 0.25*j.

Basis evaluation uses the bounded symmetric form (exact identity, no
catastrophic cancellation, so the whole chain runs in fp16):
  a   = |4x + 5 - g|          (ACT Abs, f32 affine)
  m   = relu(2 - a) = -(min(a,2) - 2)
  n   = relu(1 - a) = -(min(a,1) - 1)
  6*B_g(x) = m^3 - 4*n^3
On device we carry u = -m, w = -n (one tensor_scalar each, 4x fp16 DVE
mode) and emit Bneg = -(6B) = m2*u - (Square(2w))*w with m2 = u*u.
Bases are quantized to fp8e4 and the spline einsum runs as fp8 DoubleRow
matmuls (K=256/instr, 2x PE rate); weights are host-scaled by 512/6 with
sign folded (-coeff), and the PSUM drain applies the 1/512.
The silu base matmul stays bf16 (it carries ~96% of the output norm) and
accumulates into the same PSUM banks with sbt host-scaled by 512; it is
issued FIRST per half so the PE tail is only the last spline k-tiles.

In-dim tiles are processed in PAIRS: the Abs/Silu ACT ops run once per
pair over [128, 1024] (halves ACT op overhead), and the two odd g=10
channels of a pair form one extra DoubleRow k-pair (4D b8/w8 tiles), so
all spline matmuls run in DoubleRow mode. SBUF pressure is handled by
rotating {a, T1} through one bufs=2 pool (so the next pair's Abs only
waits on the clamps of the current pair) and {m2, T2} through a bufs=1
pool (all strictly write-after-read safe, same-engine ordered).
"""

import numpy as np
import ml_dtypes

import concourse.bass as bass
import concourse.mybir as mybir
import concourse.tile as tile
from concourse import bacc
from concourse.alu_op_type import AluOpType as A
from concourse.bass_utils import run_bass_kernel_spmd

AF = mybir.ActivationFunctionType
F32 = mybir.dt.float32
F16 = mybir.dt.float16
BF16 = mybir.dt.bfloat16
F8 = mybir.dt.float8e4
DR = mybir.MatmulPerfMode.DoubleRow

# problem constants (hardcoded per the task contract)
TOKENS, IN_DIM, OUT_DIM = 8192, 1024, 1024
NB = 11  # cubic B-spline bases per input dim (grid_size + k)
N_CORES = 8
TPC = TOKENS // N_CORES  # tokens per core (1024)
HALF = 512  # tokens per processing chunk (PSUM-bank limited)
NIT = IN_DIM // 128  # in-dim tiles (8)
NPAIR = NIT // 2  # in-dim tile pairs (4)
M_TILES = HALF // 128  # token tiles per half (4)
N_OC = OUT_DIM // 512  # out-dim chunks (2)
WOC = NB * 512  # weight free size per (it, oc) chunk (5632)
WSCALE = 512.0  # fp8 weight scale (power of 2; undone in the PSUM drain)
NCONV_ACT = 8  # basis channels converted fp16->fp8 on ACT (rest on DVE)

_CACHED = None


def _silu_matmuls(nc, psums, silu_pair, sbtts, first_pair):
    for j in range(2):
        for oc in range(N_OC):
            for m in range(M_TILES):
                nc.tensor.matmul(
                    psums[oc][m],
                    lhsT=silu_pair[:, j, m * 128:m * 128 + 128],
                    rhs=sbtts[j][:, oc * 512:(oc + 1) * 512],
                    start=(first_pair and j == 0), stop=False)


def _spline_matmuls(nc, psums, b8, w8t, oc, last_pair, m_tiles):
    # b8: [128, 11, 2, 512] (channel, it-in-pair, token)
    # w8t: [128, 2, 11, 512] (it-in-pair, channel, out-chunk)
    for m in m_tiles:
        ms = slice(m * 128, m * 128 + 128)
        for j in range(2):
            for gp in range(5):
                nc.tensor.matmul(
                    psums[oc][m],
                    lhsT=b8[:, 2 * gp:2 * gp + 2, j, ms],
                    rhs=w8t[:, j, 2 * gp:2 * gp + 2, :],
                    start=False, stop=False, perf_mode=DR)
        # the two g=10 channels of the pair form one k-pair
        nc.tensor.matmul(
            psums[oc][m], lhsT=b8[:, 10, 0:2, ms],
            rhs=w8t[:, 0:2, 10, :],
            start=False, stop=last_pair, perf_mode=DR)


def _build_bass():
    nc = bacc.Bacc("TRN2", target_bir_lowering=False, debug=False,
                   num_devices=N_CORES)
    xt = nc.declare_dram_parameter("xt", [IN_DIM, TPC], F16, isOutput=False)
    w8 = nc.declare_dram_parameter("w8", [IN_DIM, NB * OUT_DIM], F8,
                                   isOutput=False)
    sbt = nc.declare_dram_parameter("sbt", [IN_DIM, OUT_DIM], BF16,
                                    isOutput=False)
    y = nc.declare_dram_parameter("y", [TPC, OUT_DIM], F32, isOutput=True)

    with tile.TileContext(nc) as tc:
        with (
            tc.tile_pool(name="xts", bufs=2) as xpool,
            tc.tile_pool(name="silu", bufs=2) as spool,
            tc.tile_pool(name="at", bufs=2) as atpool,    # a -> T1 rotation
            tc.tile_pool(name="mt", bufs=1) as mtpool,    # m2 -> T2 rotation
            tc.tile_pool(name="un", bufs=1) as upool,
            tc.tile_pool(name="n2", bufs=1) as n2pool,
            tc.tile_pool(name="wb", bufs=1) as wpool_,
            tc.tile_pool(name="bn", bufs=1) as bnpool,
            tc.tile_pool(name="b8b", bufs=2) as b8pool,
            tc.tile_pool(name="w8t", bufs=2) as w8pool,
            tc.tile_pool(name="sbts", bufs=2) as sbpool,
            tc.tile_pool(name="outs", bufs=1) as opool,
            tc.tile_pool(name="consts", bufs=1) as kpool,
            tc.tile_pool(name="psum", bufs=8, space="PSUM") as ppool,
        ):
            # per-channel bias constants for a_g = |4x + (5-g)|
            bias_tile = kpool.tile([128, NB + 1], F32, tag="bias")
            for g in range(NB):
                nc.vector.memset(bias_tile[:, g:g + 1], float(5 - g))
            nc.vector.memset(bias_tile[:, NB:NB + 1], 0.0)  # zero bias
            zb = bias_tile[:, NB:NB + 1]

            for half in range(2):
                t0 = half * HALF
                psums = [[ppool.tile([128, 512], F32, tag="ps",
                                     name=f"ps_{half}_{_oc}_{_m}")
                          for _m in range(M_TILES)] for _oc in range(N_OC)]
                for pair in range(NPAIR):
                    it0 = 2 * pair
                    # x for both its of the pair: [128, 2, 512] f32
                    xp = xpool.tile([128, 2, HALF], F16, tag="xt")
                    for j in range(2):
                        for q in range(2):
                            nc.sync.dma_start(
                                out=xp[:, j, q * 256:(q + 1) * 256],
                                in_=xt[(it0 + j) * 128:(it0 + j + 1) * 128,
                                       t0 + q * 256:t0 + (q + 1) * 256])
                    sp = spool.tile([128, 2, HALF], BF16, tag="silu")
                    nc.scalar.activation(sp, xp, AF.Silu)
                    sbtts = []
                    for j in range(2):
                        sbtt = sbpool.tile([128, OUT_DIM], BF16, tag="sbt")
                        nc.sync.dma_start(
                            out=sbtt,
                            in_=sbt[(it0 + j) * 128:(it0 + j + 1) * 128, :])
                        sbtts.append(sbtt)
                    _silu_matmuls(nc, psums, sp, sbtts, pair == 0)

                    # 11 abs channels for the pair -> [128, 11, 2, 512] fp16
                    a = atpool.tile([128, NB, 2, HALF], F16, tag="at")
                    for g in range(NB):
                        nc.scalar.activation(a[:, g, :, :], xp, AF.Abs,
                                             bias=bias_tile[:, g:g + 1],
                                             scale=4.0)
                    # w = min(a,1)-1 = -n first (ACT Square waits on it),
                    # then u = min(a,2)-2 = -m   (tensor_scalar, 4x)
                    w = wpool_.tile([128, NB, 2, HALF], F16, tag="wb")
                    nc.vector.tensor_scalar(w, a, 1.0, 1.0, A.min, A.subtract)
                    u = upool.tile([128, NB, 2, HALF], F16, tag="un")
                    nc.vector.tensor_scalar(u, a, 2.0, 2.0, A.min, A.subtract)
                    # n2q = (2w)^2 = 4n^2 (ACT) ; m2 = u*u ; T1 = -m^3
                    n2q = n2pool.tile([128, NB, 2, HALF], F16, tag="n2")
                    nc.scalar.activation(n2q, w, AF.Square, bias=zb,
                                         scale=2.0)
                    m2 = mtpool.tile([128, NB, 2, HALF], F16, tag="mt")
                    nc.vector.tensor_mul(m2, u, u)
                    T1 = atpool.tile([128, NB, 2, HALF], F16, tag="at")
                    nc.vector.tensor_mul(T1, m2, u)
                    T2 = mtpool.tile([128, NB, 2, HALF], F16, tag="mt")
                    nc.vector.tensor_mul(T2, n2q, w)
                    # Bneg = T1 - T2 = -(6B), chunked by token-half so the
                    # PE can start each pair's matmuls before the full
                    # conversion lands. Channels < NCONV_ACT go through
                    # fp16 + ACT convert, the rest are written straight to
                    # fp8 by the DVE subtract (1x mode).
                    b8 = b8pool.tile([128, NB, 2, HALF], F8, tag="b8")
                    Bneg = bnpool.tile([128, NCONV_ACT, 2, HALF], F16,
                                       tag="bn")
                    w8ts = []
                    for oc in range(N_OC):
                        w8t = w8pool.tile([128, 2, NB, 512], F8, tag="w8")
                        for j in range(2):
                            nc.sync.dma_start(
                                out=w8t[:, j, :, :],
                                in_=w8[(it0 + j) * 128:(it0 + j + 1) * 128,
                                       oc * WOC:(oc + 1) * WOC])
                        w8ts.append(w8t)
                    for mc in range(2):
                        sl = slice(mc * 256, (mc + 1) * 256)
                        nc.vector.tensor_sub(Bneg[:, :, :, sl],
                                             T1[:, :NCONV_ACT, :, sl],
                                             T2[:, :NCONV_ACT, :, sl])
                        nc.scalar.activation(b8[:, :NCONV_ACT, :, sl],
                                             Bneg[:, :, :, sl],
                                             AF.Copy, bias=0.0, scale=1.0)
                        nc.vector.tensor_sub(b8[:, NCONV_ACT:, :, sl],
                                             T1[:, NCONV_ACT:, :, sl],
                                             T2[:, NCONV_ACT:, :, sl])
                        for oc in range(N_OC):
                            _spline_matmuls(nc, psums, b8, w8ts[oc], oc,
                                            pair == NPAIR - 1,
                                            (2 * mc, 2 * mc + 1))
                # drain PSUM -> SBUF (undo WSCALE) -> HBM
                for oc in range(N_OC):
                    for m in range(M_TILES):
                        ot = opool.tile([128, 512], F32, tag="out")
                        nc.scalar.activation(ot, psums[oc][m], AF.Copy,
                                             bias=0.0, scale=1.0 / WSCALE)
                        r0 = t0 + m * 128
                        nc.sync.dma_start(
                            out=y[r0:r0 + 128, oc * 512:(oc + 1) * 512],
                            in_=ot)
    nc.compile()
    return nc


def _prepare_inputs(x, coeff, scale_base, scale_spline):
    x = np.asarray(x, dtype=np.float32)
    coeff = np.asarray(coeff, dtype=np.float32)
    scale_base = np.asarray(scale_base, dtype=np.float32)
    ss = float(np.asarray(scale_spline).reshape(-1)[0])
    # w8[i, oc*5632 + g*512 + o] = -coeff[oc*512+o, i, g] * ss * WSCALE/6
    w8 = (coeff * (-ss * WSCALE / 6.0)).transpose(1, 2, 0)  # [i, g, o]
    w8 = w8.reshape(IN_DIM, NB, N_OC, 512).transpose(0, 2, 1, 3)
    w8 = np.ascontiguousarray(w8).reshape(IN_DIM, NB * OUT_DIM)
    w8 = w8.astype(ml_dtypes.float8_e4m3)
    sbt = np.ascontiguousarray(scale_base.T * WSCALE).astype(
        ml_dtypes.bfloat16)
    in_maps = []
    for c in range(N_CORES):
        xt = np.ascontiguousarray(x[c * TPC:(c + 1) * TPC, :].T).astype(
            np.float16)
        in_maps.append({"xt": xt, "w8": w8, "sbt": sbt})
    return in_maps


def _get_bass():
    global _CACHED
    if _CACHED is None:
        _CACHED = _build_bass()
    return _CACHED


def run(inputs, trace=False, **kw):
    nc = _get_bass()
    in_maps = _prepare_inputs(inputs["x"], inputs["coeff"],
                              inputs["scale_base"], inputs["scale_spline"])
    res = run_bass_kernel_spmd(nc, in_maps, list(range(N_CORES)),
                               trace=trace, **kw)
    y = np.concatenate([np.asarray(res.results[c]["y"])
                        for c in range(N_CORES)], axis=0)
    return np.ascontiguousarray(y.astype(np.float32)), res


def kernel(x, grid, coeff, scale_base, scale_spline):
    y, _ = run({"x": x, "grid": grid, "coeff": coeff,
                "scale_base": scale_base, "scale_spline": scale_spline})
    return y
